# revision 12
# baseline (speedup 1.0000x reference)
"""GAT kernel for Trainium2 (Bass/Tile), data-parallel over batch on 8 cores.

Per-core math (one batch element, N nodes, H heads, D=E=128). Softmax is
invariant to per-row scaling, so the row factor of the exact exp'd scores
cancels and the attention weights can be computed as

  p[j,i] = max(m2_j, u3_i) * adjT[j,i],  feat scaled per-row by s23_j
    u3_i  = exp(0.8*a_s_i)        (host)
    m2_j  = exp(-0.8*a_n_j)       (host)
    s23_j = exp(a_n_j - 2)        (host)

since exp(lrelu(z,0.2)) = exp(0.2 z)*exp(0.8 relu(z)); dropping the
cancelling exp(0.8 a_s_i) row factor the weight is s23_j*max(m2_j, u3_i),
and the s23_j factor commutes through the j-contraction onto feat (and the
ones/rowsum column), leaving a single fused DVE op for the scores.

On device, per (head, j-chunk):
  p = scalar_tensor_tensor(U3B, m2_chunk, adjT_chunk, max, mult)  (DVE, 2x)
  acc_ib += p[:, ib-block]^T @ [s23*feat | s23]  for all 8 i-blocks (PE,
            bf16, incremental PSUM accumulation; the weighted rowsum falls
            out of the s23 column)
  out[i,:] = relu(acc * (1/rowsum))   (ACT relu with per-partition scale,
            reciprocal on DVE; feat scaling rides free on the ACT
            PSUM->SBUF feat copies as a per-partition scale)
No N^2 work ever touches the scalar engine and nothing round-trips PSUM
except the matmul accumulators themselves.
"""

import os
import sys

sys.path.insert(0, "/opt/trn_rl_repo")

import numpy as np
import ml_dtypes

import concourse.bass as bass
import concourse.bacc as bacc
import concourse.mybir as mybir
import concourse.tile as tile
from concourse.bass_utils import run_bass_kernel_spmd

F32 = mybir.dt.float32
F16 = mybir.dt.bfloat16
NP16 = ml_dtypes.bfloat16
P = 128


def build_core_program(N, H, D=128, E=128):
    """Trace the Bass program computing one batch element of the GAT."""
    nc = bacc.Bacc("TRN2", debug=False, target_bir_lowering=False)
    NCH = N // P   # node chunks
    EA = E + 1     # feat columns + ones column
    HP = H // 2    # head pairs

    # wx = [kw (H*E cols, head-major) | xT (N cols)] packed on host
    WXW = H * E + N
    wx = nc.dram_tensor("wx", [D, WXW], F16, kind="ExternalInput").ap()
    adjT = nc.dram_tensor("adjT", [N, N], F16, kind="ExternalInput").ap()
    # u3 rows broadcast across partitions, head-major blocks of 128 rows
    u3b = nc.dram_tensor("u3b", [H * P, N], F16, kind="ExternalInput").ap()
    # per-partition scalars: col c*H*2 + h*2 = m2, +1 = s23 (fp32)
    svt = nc.dram_tensor("svt", [P, NCH * H * 2], F32,
                         kind="ExternalInput").ap()
    out = nc.dram_tensor("out", [N, H * E], F32, kind="ExternalOutput").ap()

    with tile.TileContext(nc) as tc:
        with (
            tc.tile_pool(name="xt", bufs=1) as xt_pool,
            tc.tile_pool(name="u3b", bufs=1) as u3b_pool,
            tc.tile_pool(name="svt", bufs=1) as svt_pool,
            tc.tile_pool(name="adj", bufs=1) as adj_pool,
            tc.tile_pool(name="fr", bufs=1) as fr_pool,
        ):
            wx_sb = xt_pool.tile([D, WXW], F16, tag="wx")
            kw_sb = wx_sb[:, 0:H * E]
            xt_sb = wx_sb[:, H * E:WXW]
            nc.sync.dma_start(out=wx_sb[:, 0:H * E + N // 2],
                              in_=wx[:, 0:H * E + N // 2])
            nc.sync.dma_start(out=wx_sb[:, H * E + N // 2:WXW],
                              in_=wx[:, H * E + N // 2:WXW])

            svt_sb = svt_pool.tile([P, NCH * H * 2], F32, tag="svt")
            nc.sync.dma_start(out=svt_sb[:], in_=svt[:])

            # DMA order tuned for startup: head 0 needs u3b[0] and early adj
            # chunks first; later heads' u3b rows and adj chunks can trail.
            u3b_sb = [u3b_pool.tile([P, N], F16, tag=f"u3b{h}",
                                    name=f"u3b{h}") for h in range(H)]
            adj_sb = [adj_pool.tile([P, N], F16, tag=f"adj{c}",
                                    name=f"adj{c}") for c in range(NCH)]
            nc.sync.dma_start(out=u3b_sb[0][:], in_=u3b[0:P, :])
            for c in range(NCH):
                nc.sync.dma_start(out=adj_sb[c][:],
                                  in_=adjT[c * P:(c + 1) * P, :])
                if c < H - 1:
                    nc.sync.dma_start(out=u3b_sb[c + 1][:],
                                      in_=u3b[(c + 1) * P:(c + 2) * P, :])

            # fs[h][c]: [P, E+1] bf16 = [s23*feat_h | s23] (scaled feat +
            # weighted-rowsum column)
            fs = [[fr_pool.tile([P, EA], F16, tag=f"fs{h}_{c}",
                                name=f"fs{h}_{c}")
                   for c in range(NCH)] for h in range(H)]

            # ---- PSUM: proj 1 bank + 7 accumulator banks = 8 (bufs is
            # per-tag, so acc_ps holds NCH-1 single-buffered slots).
            with (
                tc.tile_pool(name="proj_ps", bufs=1, space="PSUM") as proj_ps,
                tc.tile_pool(name="acc_ps", bufs=1, space="PSUM") as acc_ps,
            ):
                # projection: all H heads in one rhs (H*E <= 512 cols)
                for c in range(NCH):
                    ps = proj_ps.tile([P, H * E], F32, tag="proj")
                    nc.tensor.matmul(
                        ps[:],
                        xt_sb[:, c * P:(c + 1) * P],
                        kw_sb,
                        start=True, stop=True,
                    )
                    for h in range(H):
                        s23 = svt_sb[:, c * H * 2 + h * 2 + 1:
                                     c * H * 2 + h * 2 + 2]
                        nc.scalar.activation(
                            fs[h][c][:, 0:E], ps[:, h * E:(h + 1) * E],
                            mybir.ActivationFunctionType.Copy,
                            bias=0.0, scale=s23)
                        nc.scalar.activation(
                            fs[h][c][:, E:E + 1], s23,
                            mybir.ActivationFunctionType.Copy)

                # ---- per-head attention ----
                with (
                    tc.tile_pool(name="p", bufs=4) as p_pool,
                    tc.tile_pool(name="ep", bufs=4) as ep_pool,
                ):
                    for h in range(H):
                        # NCH concurrent accumulators: i-blocks 0..NCH-1
                        accs = []
                        for ib in range(NCH):
                            if ib < NCH - 1:
                                accs.append(acc_ps.tile(
                                    [P, EA], F32, tag=f"acc{ib}",
                                    name=f"acc{h}_{ib}"))
                            else:
                                accs.append(proj_ps.tile(
                                    [P, EA], F32, tag="proj",
                                    name=f"acc{h}_{ib}"))
                        for c in range(NCH):
                            sc = c * H * 2 + h * 2
                            p = p_pool.tile([P, N], F16, tag="p",
                                            name=f"p{h}_{c}")
                            nc.vector.scalar_tensor_tensor(
                                out=p[:],
                                in0=u3b_sb[h][:],
                                scalar=svt_sb[:, sc:sc + 1],
                                in1=adj_sb[c][:],
                                op0=mybir.AluOpType.max,
                                op1=mybir.AluOpType.mult)
                            for ib in range(NCH):
                                nc.tensor.matmul(
                                    accs[ib][:],
                                    p[:, ib * P:(ib + 1) * P],
                                    fs[h][c][:],
                                    start=(c == 0), stop=(c == NCH - 1),
                                )

                        # epilogue: out = relu(acc / rowsum), ones-col rowsum
                        HB = NCH // 2
                        for ib in range(NCH):
                            acc = accs[ib]
                            rec = ep_pool.tile([P, 1], F32, tag="rec",
                                               name=f"rec{h}_{ib}")
                            nc.vector.reciprocal(rec[:], acc[:, E:E + 1])
                            if ib == 0:
                                obh = [ep_pool.tile([P, HB * E], F32,
                                                    tag=f"obh{half}", bufs=2,
                                                    name=f"obh{h}_{half}")
                                       for half in range(2)]
                            nc.scalar.activation(
                                obh[ib // HB][:, (ib % HB) * E:
                                              (ib % HB + 1) * E],
                                acc[:, 0:E],
                                mybir.ActivationFunctionType.Relu,
                                bias=0.0, scale=rec[:])
                        # two DMAs per head (i-block halves):
                        # partition r, free (ib, c) -> row ib*P+r, col h*E+c
                        for half in range(2):
                            nc.sync.dma_start(
                                out=out[half * HB * P:(half + 1) * HB * P,
                                        h * E:(h + 1) * E].rearrange(
                                    "(ib r) c -> r ib c", r=P),
                                in_=obh[half][:].rearrange(
                                    "p (ib c) -> p ib c", c=E))
    nc.compile()
    return nc


_PROGRAM_CACHE = {}


def _get_program(N, H):
    key = (N, H)
    if key not in _PROGRAM_CACHE:
        _PROGRAM_CACHE[key] = build_core_program(N, H)
    return _PROGRAM_CACHE[key]


def host_prep(x, adj, kernel, attn_self, attn_neigh):
    """Build per-core input maps (layout transforms + small vector math)."""
    B, N, D = x.shape
    H, _, E = kernel.shape
    NCH = N // P
    kas = np.stack([kernel[h] @ attn_self[h] for h in range(H)], 1)  # [D,H]
    kan = np.stack([kernel[h] @ attn_neigh[h] for h in range(H)], 1)
    kw = np.concatenate([kernel[h] for h in range(H)], axis=1)  # [D, H*E]
    in_maps = []
    for b in range(B):
        a_s = x[b] @ kas   # [N, H]
        a_n = x[b] @ kan
        u3 = np.exp(0.8 * a_s.T)                  # [H, N]
        m2 = np.exp(-0.8 * a_n.T)
        s23 = np.exp(a_n.T - 2.0)
        u3b = np.repeat(u3.astype(NP16), P, axis=0)   # [H*P, N] broadcast
        svt = np.empty((P, NCH * H * 2), np.float32)
        for c in range(NCH):
            for h in range(H):
                svt[:, c * H * 2 + h * 2] = m2[h, c * P:(c + 1) * P]
                svt[:, c * H * 2 + h * 2 + 1] = s23[h, c * P:(c + 1) * P]
        wx = np.concatenate([kw, x[b].T], axis=1)
        in_maps.append({
            "wx": np.ascontiguousarray(wx).astype(NP16),
            "adjT": np.ascontiguousarray(adj[b].T).astype(NP16),
            "u3b": np.ascontiguousarray(u3b),
            "svt": svt,
        })
    return in_maps


def kernel(x, adj, kernel, attn_self, attn_neigh, bias, _profile=None):
    x = np.asarray(x, np.float32)
    adj = np.asarray(adj, np.float32)
    kernel = np.asarray(kernel, np.float32)
    attn_self = np.asarray(attn_self, np.float32)
    attn_neigh = np.asarray(attn_neigh, np.float32)
    bias = np.asarray(bias, np.float32)

    B, N, D = x.shape
    H, _, E = kernel.shape
    nc = _get_program(N, H)
    in_maps = host_prep(x, adj, kernel, attn_self, attn_neigh)
    kwargs = dict(_profile) if _profile else {}
    last_err = None
    for _attempt in range(3):
        try:
            res = run_bass_kernel_spmd(nc, in_maps, list(range(B)), **kwargs)
            outs = np.stack(
                [np.asarray(res.results[b]["out"]) for b in range(B)])
            break
        except Exception as exc:  # transient PJRT/axon fetch errors
            last_err = exc
    else:
        raise last_err
    assert not np.any(bias != 0.0), "nonzero-bias path not implemented"
    if _profile:
        return outs, res
    return outs


if __name__ == "__main__":
    # Mini smoke test: N=256, H=2, B=2 against a numpy reference.
    np.random.seed(0)
    N, H, D, E, B = 256, 2, 128, 128, 2
    x = np.random.randn(B, N, D).astype(np.float32)
    adj = (np.random.rand(B, N, N) < 0.5).astype(np.float32)
    K = (np.random.randn(H, D, E) / np.sqrt(D)).astype(np.float32)
    a_s = (np.random.randn(H, E) / np.sqrt(E)).astype(np.float32)
    a_n = (np.random.randn(H, E) / np.sqrt(E)).astype(np.float32)
    bias = np.zeros((H, E), np.float32)

    def ref(x, adj, K, a_s, a_n, bias):
        feat = np.einsum('bnd,hde->bhne', x, K)
        s1 = np.einsum('bhne,he->bhn', feat, a_s)
        s2 = np.einsum('bhne,he->bhn', feat, a_n)
        sc = s1[..., :, None] + s2[..., None, :]
        sc = np.where(sc > 0, sc, 0.2 * sc)
        sc = sc + (-1e10) * (1.0 - adj[:, None])
        sc = sc - sc.max(axis=-1, keepdims=True)
        att = np.exp(sc)
        att = att / att.sum(axis=-1, keepdims=True)
        o = np.einsum('bhnm,bhme->bhne', att, feat) + bias[None, :, None, :]
        o = o.transpose(0, 2, 1, 3).reshape(B, N, H * E)
        return np.maximum(o, 0.0)

    expected = ref(x, adj, K, a_s, a_n, bias)
    nc = _get_program(N, H)
    in_maps = host_prep(x, adj, K, a_s, a_n)
    res = run_bass_kernel_spmd(nc, in_maps, list(range(B)))
    actual = np.stack([np.asarray(res.results[b]["out"]) for b in range(B)])
    err = np.abs(actual - expected).max() / np.abs(expected).max()
    rel = np.linalg.norm(actual - expected) / np.linalg.norm(expected)
    print(f"SMOKE absmax-rel: {err:.3e}  l2-rel: {rel:.3e}")


# revision 13
# speedup vs baseline: 1.0513x; 1.0513x over previous
"""GAT kernel for Trainium2 (Bass/Tile), data-parallel over batch on 8 cores.

Per-core math (one batch element, N nodes, H heads, D=E=128). Softmax is
invariant to per-row scaling, so the row factor of the exact exp'd scores
cancels and the attention weights can be computed as

  p[j,i] = max(m2_j, u3_i) * adjT[j,i],  feat scaled per-row by s23_j
    u3_i  = exp(0.8*a_s_i)        (host)
    m2_j  = exp(-0.8*a_n_j)       (host)
    s23_j = exp(a_n_j - 2)        (host)

since exp(lrelu(z,0.2)) = exp(0.2 z)*exp(0.8 relu(z)); dropping the
cancelling exp(0.8 a_s_i) row factor the weight is s23_j*max(m2_j, u3_i),
and the s23_j factor commutes through the j-contraction onto feat (and the
ones/rowsum column), leaving a single fused DVE op for the scores.

On device, per (head, j-chunk):
  p = scalar_tensor_tensor(U3B, m2_chunk, adjT_chunk, max, mult)  (DVE, 2x)
  acc_ib += p[:, ib-block]^T @ [s23*feat | s23]  for all 8 i-blocks (PE,
            bf16, incremental PSUM accumulation; the weighted rowsum falls
            out of the s23 column)
  out[i,:] = relu(acc * (1/rowsum))   (ACT relu with per-partition scale,
            reciprocal on DVE; feat scaling rides free on the ACT
            PSUM->SBUF feat copies as a per-partition scale)
No N^2 work ever touches the scalar engine and nothing round-trips PSUM
except the matmul accumulators themselves.
"""

import os
import sys

sys.path.insert(0, "/opt/trn_rl_repo")

import numpy as np
import ml_dtypes

import concourse.bass as bass
import concourse.bacc as bacc
import concourse.mybir as mybir
import concourse.tile as tile
from concourse.bass_utils import run_bass_kernel_spmd

F32 = mybir.dt.float32
F16 = mybir.dt.bfloat16
NP16 = ml_dtypes.bfloat16
P = 128


def build_core_program(N, H, D=128, E=128):
    """Trace the Bass program computing one batch element of the GAT."""
    nc = bacc.Bacc("TRN2", debug=False, target_bir_lowering=False)
    NCH = N // P   # node chunks
    EA = E + 1     # feat columns + ones column
    HP = H // 2    # head pairs

    # wx = [kw (H*E cols, head-major) | xT (N cols)] packed on host
    WXW = H * E + N
    wx = nc.dram_tensor("wx", [D, WXW], F16, kind="ExternalInput").ap()
    adjT = nc.dram_tensor("adjT", [N, N], F16, kind="ExternalInput").ap()
    # u3 rows broadcast across partitions, head-major blocks of 128 rows
    u3b = nc.dram_tensor("u3b", [H * P, N], F16, kind="ExternalInput").ap()
    # per-partition scalars: m2 bf16 (STT max scalar, col c*H+h), s23 fp32
    # (ACT feat scale / rowsum column)
    svm = nc.dram_tensor("svm", [P, NCH * H], F16, kind="ExternalInput").ap()
    svs = nc.dram_tensor("svs", [P, NCH * H], F32, kind="ExternalInput").ap()
    out = nc.dram_tensor("out", [N, H * E], F32, kind="ExternalOutput").ap()

    with tile.TileContext(nc) as tc:
        with (
            tc.tile_pool(name="xt", bufs=1) as xt_pool,
            tc.tile_pool(name="u3b", bufs=1) as u3b_pool,
            tc.tile_pool(name="svt", bufs=1) as svt_pool,
            tc.tile_pool(name="adj", bufs=1) as adj_pool,
            tc.tile_pool(name="fr", bufs=1) as fr_pool,
        ):
            wx_sb = xt_pool.tile([D, WXW], F16, tag="wx")
            kw_sb = wx_sb[:, 0:H * E]
            xt_sb = wx_sb[:, H * E:WXW]
            nc.sync.dma_start(out=wx_sb[:, 0:H * E + N // 2],
                              in_=wx[:, 0:H * E + N // 2])
            nc.sync.dma_start(out=wx_sb[:, H * E + N // 2:WXW],
                              in_=wx[:, H * E + N // 2:WXW])

            svm_sb = svt_pool.tile([P, NCH * H], F16, tag="svm")
            nc.sync.dma_start(out=svm_sb[:], in_=svm[:])
            svs_sb = svt_pool.tile([P, NCH * H], F32, tag="svs")
            nc.sync.dma_start(out=svs_sb[:], in_=svs[:])

            # DMA order tuned for startup: head 0 needs u3b[0] and early adj
            # chunks first; later heads' u3b rows and adj chunks can trail.
            u3b_sb = [u3b_pool.tile([P, N], F16, tag=f"u3b{h}",
                                    name=f"u3b{h}") for h in range(H)]
            adj_sb = [adj_pool.tile([P, N], F16, tag=f"adj{c}",
                                    name=f"adj{c}") for c in range(NCH)]
            nc.sync.dma_start(out=u3b_sb[0][:], in_=u3b[0:P, :])
            for c in range(NCH):
                nc.sync.dma_start(out=adj_sb[c][:],
                                  in_=adjT[c * P:(c + 1) * P, :])
                if c < H - 1:
                    nc.sync.dma_start(out=u3b_sb[c + 1][:],
                                      in_=u3b[(c + 1) * P:(c + 2) * P, :])

            # fs[c]: [P, H*(E+1)] bf16, head-major [s23*feat_h | s23] blocks
            # (scaled feat + weighted-rowsum column per head)
            fs = [fr_pool.tile([P, H * EA], F16, tag=f"fs{c}", name=f"fs{c}")
                  for c in range(NCH)]

            # ---- PSUM: proj 1 bank + 7 accumulator banks = 8 (bufs is
            # per-tag, so acc_ps holds NCH-1 single-buffered slots).
            with (
                tc.tile_pool(name="proj_ps", bufs=1, space="PSUM") as proj_ps,
                tc.tile_pool(name="acc_ps", bufs=1, space="PSUM") as acc_ps,
            ):
                # projection: all H heads in one rhs (H*E <= 512 cols)
                for c in range(NCH):
                    ps = proj_ps.tile([P, H * E], F32, tag="proj")
                    nc.tensor.matmul(
                        ps[:],
                        xt_sb[:, c * P:(c + 1) * P],
                        kw_sb,
                        start=True, stop=True,
                    )
                    for h in range(H):
                        s23 = svs_sb[:, c * H + h:c * H + h + 1]
                        nc.scalar.activation(
                            fs[c][:, h * EA:h * EA + E],
                            ps[:, h * E:(h + 1) * E],
                            mybir.ActivationFunctionType.Copy,
                            bias=0.0, scale=s23)
                    # all H rowsum columns (= s23) in one strided copy
                    nc.vector.tensor_copy(
                        fs[c][:].rearrange("p (h f) -> p h f", h=H)[:, :, E],
                        svs_sb[:, c * H:(c + 1) * H])

                # ---- per-head attention ----
                with (
                    tc.tile_pool(name="p", bufs=4) as p_pool,
                    tc.tile_pool(name="ep", bufs=4) as ep_pool,
                ):
                    for h in range(H):
                        # NCH concurrent accumulators: i-blocks 0..NCH-1
                        accs = []
                        for ib in range(NCH):
                            if ib < NCH - 1:
                                accs.append(acc_ps.tile(
                                    [P, EA], F32, tag=f"acc{ib}",
                                    name=f"acc{h}_{ib}"))
                            else:
                                accs.append(proj_ps.tile(
                                    [P, EA], F32, tag="proj",
                                    name=f"acc{h}_{ib}"))
                        for c in range(NCH):
                            sc = c * H + h
                            p = p_pool.tile([P, N], F16, tag="p",
                                            name=f"p{h}_{c}")
                            nc.vector.scalar_tensor_tensor(
                                out=p[:],
                                in0=u3b_sb[h][:],
                                scalar=svm_sb[:, sc:sc + 1],
                                in1=adj_sb[c][:],
                                op0=mybir.AluOpType.max,
                                op1=mybir.AluOpType.mult)
                            for ib in range(NCH):
                                nc.tensor.matmul(
                                    accs[ib][:],
                                    p[:, ib * P:(ib + 1) * P],
                                    fs[c][:, h * EA:(h + 1) * EA],
                                    start=(c == 0), stop=(c == NCH - 1),
                                )

                        # epilogue: out = relu(acc / rowsum), ones-col rowsum
                        HB = NCH // 2
                        for ib in range(NCH):
                            acc = accs[ib]
                            rec = ep_pool.tile([P, 1], F32, tag="rec",
                                               name=f"rec{h}_{ib}")
                            nc.vector.reciprocal(rec[:], acc[:, E:E + 1])
                            if ib == 0:
                                obh = [ep_pool.tile([P, HB * E], F32,
                                                    tag=f"obh{half}", bufs=2,
                                                    name=f"obh{h}_{half}")
                                       for half in range(2)]
                            nc.scalar.activation(
                                obh[ib // HB][:, (ib % HB) * E:
                                              (ib % HB + 1) * E],
                                acc[:, 0:E],
                                mybir.ActivationFunctionType.Relu,
                                bias=0.0, scale=rec[:])
                        # two DMAs per head (i-block halves):
                        # partition r, free (ib, c) -> row ib*P+r, col h*E+c
                        for half in range(2):
                            nc.sync.dma_start(
                                out=out[half * HB * P:(half + 1) * HB * P,
                                        h * E:(h + 1) * E].rearrange(
                                    "(ib r) c -> r ib c", r=P),
                                in_=obh[half][:].rearrange(
                                    "p (ib c) -> p ib c", c=E))
    nc.compile()
    return nc


_PROGRAM_CACHE = {}


def _get_program(N, H):
    key = (N, H)
    if key not in _PROGRAM_CACHE:
        _PROGRAM_CACHE[key] = build_core_program(N, H)
    return _PROGRAM_CACHE[key]


def host_prep(x, adj, kernel, attn_self, attn_neigh):
    """Build per-core input maps (layout transforms + small vector math)."""
    B, N, D = x.shape
    H, _, E = kernel.shape
    NCH = N // P
    kas = np.stack([kernel[h] @ attn_self[h] for h in range(H)], 1)  # [D,H]
    kan = np.stack([kernel[h] @ attn_neigh[h] for h in range(H)], 1)
    kw = np.concatenate([kernel[h] for h in range(H)], axis=1)  # [D, H*E]
    in_maps = []
    for b in range(B):
        a_s = x[b] @ kas   # [N, H]
        a_n = x[b] @ kan
        u3 = np.exp(0.8 * a_s.T)                  # [H, N]
        m2 = np.exp(-0.8 * a_n.T)
        s23 = np.exp(a_n.T - 2.0)
        u3b = np.repeat(u3.astype(NP16), P, axis=0)   # [H*P, N] broadcast
        # svm[r, c*H+h] = m2[h, c*128+r]; svs likewise for s23
        svm = m2.T.reshape(NCH, P, H).transpose(1, 0, 2).reshape(P, NCH * H)
        svs = s23.T.reshape(NCH, P, H).transpose(1, 0, 2).reshape(P, NCH * H)
        wx = np.concatenate([kw, x[b].T], axis=1)
        in_maps.append({
            "wx": np.ascontiguousarray(wx).astype(NP16),
            "adjT": np.ascontiguousarray(adj[b].T).astype(NP16),
            "u3b": np.ascontiguousarray(u3b),
            "svm": np.ascontiguousarray(svm).astype(NP16),
            "svs": np.ascontiguousarray(svs).astype(np.float32),
        })
    return in_maps


def kernel(x, adj, kernel, attn_self, attn_neigh, bias, _profile=None):
    x = np.asarray(x, np.float32)
    adj = np.asarray(adj, np.float32)
    kernel = np.asarray(kernel, np.float32)
    attn_self = np.asarray(attn_self, np.float32)
    attn_neigh = np.asarray(attn_neigh, np.float32)
    bias = np.asarray(bias, np.float32)

    B, N, D = x.shape
    H, _, E = kernel.shape
    nc = _get_program(N, H)
    in_maps = host_prep(x, adj, kernel, attn_self, attn_neigh)
    kwargs = dict(_profile) if _profile else {}
    last_err = None
    for _attempt in range(3):
        try:
            res = run_bass_kernel_spmd(nc, in_maps, list(range(B)), **kwargs)
            outs = np.stack(
                [np.asarray(res.results[b]["out"]) for b in range(B)])
            break
        except Exception as exc:  # transient PJRT/axon fetch errors
            last_err = exc
    else:
        raise last_err
    assert not np.any(bias != 0.0), "nonzero-bias path not implemented"
    if _profile:
        return outs, res
    return outs


if __name__ == "__main__":
    # Mini smoke test: N=256, H=2, B=2 against a numpy reference.
    np.random.seed(0)
    N, H, D, E, B = 256, 2, 128, 128, 2
    x = np.random.randn(B, N, D).astype(np.float32)
    adj = (np.random.rand(B, N, N) < 0.5).astype(np.float32)
    K = (np.random.randn(H, D, E) / np.sqrt(D)).astype(np.float32)
    a_s = (np.random.randn(H, E) / np.sqrt(E)).astype(np.float32)
    a_n = (np.random.randn(H, E) / np.sqrt(E)).astype(np.float32)
    bias = np.zeros((H, E), np.float32)

    def ref(x, adj, K, a_s, a_n, bias):
        feat = np.einsum('bnd,hde->bhne', x, K)
        s1 = np.einsum('bhne,he->bhn', feat, a_s)
        s2 = np.einsum('bhne,he->bhn', feat, a_n)
        sc = s1[..., :, None] + s2[..., None, :]
        sc = np.where(sc > 0, sc, 0.2 * sc)
        sc = sc + (-1e10) * (1.0 - adj[:, None])
        sc = sc - sc.max(axis=-1, keepdims=True)
        att = np.exp(sc)
        att = att / att.sum(axis=-1, keepdims=True)
        o = np.einsum('bhnm,bhme->bhne', att, feat) + bias[None, :, None, :]
        o = o.transpose(0, 2, 1, 3).reshape(B, N, H * E)
        return np.maximum(o, 0.0)

    expected = ref(x, adj, K, a_s, a_n, bias)
    nc = _get_program(N, H)
    in_maps = host_prep(x, adj, K, a_s, a_n)
    res = run_bass_kernel_spmd(nc, in_maps, list(range(B)))
    actual = np.stack([np.asarray(res.results[b]["out"]) for b in range(B)])
    err = np.abs(actual - expected).max() / np.abs(expected).max()
    rel = np.linalg.norm(actual - expected) / np.linalg.norm(expected)
    print(f"SMOKE absmax-rel: {err:.3e}  l2-rel: {rel:.3e}")
